# revision 53
# baseline (speedup 1.0000x reference)
"""Trainium2 Bass kernel: ViT-style dense transformer block (B=64,S=577,D=768,H=12).

Sharding: pure data-parallel over batch across 8 NeuronCores (8 batches/core,
no collectives).  Per core:

  Phase 1 (per batch): LN1 -> QKV -> attention -> out-proj + residual,
    spilling the residual stream x2 to DRAM scratch.
    - QKV and the out-projection run in fp8-e4m3 with DoubleRow perf mode
      (2 contraction rows / cycle).  Weights are pre-scaled (x16 for
      Wq/Wk/Wv with a 1/16 descale folded into the PSUM->SBUF epilogue;
      x4 for Wo with the 1/4 descale folded into the softmax denominator
      via a 4.0-valued ones column in V).
    - Scores are computed transposed (scoresT[j,i] = k_j . q_i) per head
      with K=64; the even/odd heads of a pair are issued back-to-back so
      they run concurrently in disjoint PE row-groups.
    - Softmax denominator comes from the extra 4.0 column appended to V;
      normalization = reciprocal + gpsimd partition-broadcast + multiply.
    - Biases bo (and bv) are folded into the matmul accumulation as K=1
      ones-row matmuls; bq/bk ride the PSUM->SBUF epilogues.

  Phase 2 (per 512-token chunk, last chunk 520): LN2 -> fc1 + tanh-GELU
    (scalar engine) -> fc2 + residual, all bf16 (fp8 fails the accuracy
    budget for the MLP), b2 folded into the fc2 accumulation.

PSUM plan (8 banks), phase 1: scores ring2 x 2 banks, PV accumulator
ring1 x 2 banks, and a shared ring2 of 1-bank tiles for QKV / out-proj
chunks + PE transposes — so next-batch QKV always has PSUM available
while the current batch's ACT-bound softmax runs.
"""

import math
import numpy as np

import concourse.bass as bass
import concourse.mybir as mybir
import concourse.tile as tile
from concourse.masks import make_identity

F32 = mybir.dt.float32
I32 = mybir.dt.int32
BF16 = mybir.dt.bfloat16
FP8 = mybir.dt.float8e4
AF = mybir.ActivationFunctionType
OP = mybir.AluOpType
DR = mybir.MatmulPerfMode.DoubleRow
RSQRT_MAGIC = 0x5F3759DF

B, S, D, H, DH = 64, 577, 768, 12, 64
SP = 592               # S padded so fp8 DoubleRow ko-steps are 16B-aligned
FF = 4 * D
EPS = 1e-6
NCORES = 8
KK = D // 128          # 6 k-tiles over D
G = KK // 2            # 3 DoubleRow k-groups
MFF = FF // 128        # 24 tiles over FF
NHP = H // 2           # 6 head pairs
SCALE = 1.0 / math.sqrt(DH)
WS_QKV = 16.0          # fp8 weight pre-scale for Wq/Wk/Wv
WS_O = 4.0             # fp8 weight pre-scale for Wo (descale via V ones col)

USE_FP8 = True

# token tiles within one sequence: 4 x 128 + 65
S_TILES = [(i * 128, min(128, S - i * 128)) for i in range((S + 127) // 128)]
# n-chunks over S and D for PSUM-bank-sized matmul outputs
S_CHUNKS = [(0, 512), (512, S - 512)]
D_CHUNKS = [(0, 512), (512, D - 512)]


def _bcast(ap):
    """[N] dram AP -> [128, N] partition-broadcast AP."""
    return bass.AP(tensor=ap.tensor, offset=ap.offset, ap=[[0, 128]] + list(ap.ap))


def _ln_stats_tile(nc, pool, x_sl, rows, mvb, i):
    """bn stats over the free dim (768) of x_sl[:rows] -> mvb[:, i, :]=(mu,var)."""
    stats = pool.tile([128, 3, 6], F32, tag="lnstats", name="lnstats")
    for sg in range(3):
        nc.vector.bn_stats(stats[:rows, sg, :], x_sl[:, 256 * sg:256 * (sg + 1)])
    nc.vector.bn_aggr(mvb[:rows, i, :], stats[:rows])


def _rsqrt_batch(nc, pool, mvb, n):
    """rstd[:, i] = 1/sqrt(var_i + EPS), magic-constant + 2 Newton iters on DVE."""
    veps = pool.tile([128, 8], F32, tag="lnveps", name="veps")
    nc.vector.tensor_scalar_add(veps[:, :n], mvb[:, 0:n, 1], EPS)
    hv = pool.tile([128, 8], F32, tag="lnhv", name="hv")
    nc.vector.tensor_scalar_mul(hv[:, :n], veps[:, :n], 0.5)
    y = pool.tile([128, 8], F32, tag="lnrstd", name="rstd_b")
    t = pool.tile([128, 8], F32, tag="lnnt", name="nt")
    nc.vector.tensor_scalar(t[:, :n].bitcast(I32), veps[:, :n].bitcast(I32),
                            1, None, op0=OP.arith_shift_right)
    nc.vector.tensor_scalar(y[:, :n].bitcast(I32), t[:, :n].bitcast(I32),
                            -1, RSQRT_MAGIC, op0=OP.mult, op1=OP.add)
    for _ in range(2):
        nc.vector.tensor_tensor(t[:, :n], y[:, :n], y[:, :n], OP.mult)
        nc.vector.tensor_tensor(t[:, :n], t[:, :n], hv[:, :n], OP.mult)
        nc.vector.tensor_scalar(t[:, :n], t[:, :n], -1.0, 1.5,
                                op0=OP.mult, op1=OP.add)
        nc.vector.tensor_tensor(y[:, :n], y[:, :n], t[:, :n], OP.mult)
    return y


def _ln_apply(nc, x_sl, rows, mvb, rstd_b, i, out_sl):
    """(x - mu_i) * rstd_i -> out_sl (one DVE op, per-partition scalars)."""
    nc.vector.tensor_scalar(out_sl, x_sl, mvb[:rows, i, 0:1],
                            rstd_b[:rows, i:i + 1],
                            op0=OP.subtract, op1=OP.mult)


def _transpose_to(nc, ptp, ident, src, rows, dst, col0, g_pp, b_pp, ncols):
    """src[:rows, 0:768] bf16 -> dst[:, kk, col0:col0+rows] feature-major,
    applying the LN gain/bias as per-partition scalars during the DVE copy."""
    for kk in range(KK):
        tp = ptp.tile([128, 128], BF16, tag="mm1", name="tp")
        nc.tensor.transpose(tp[:, :rows], src[:rows, 128 * kk:128 * (kk + 1)],
                            ident[:rows, :rows])
        nc.vector.tensor_scalar(dst[:, kk, col0:col0 + rows], tp[:, :rows],
                                g_pp[:, kk:kk + 1], b_pp[:, kk:kk + 1],
                                op0=OP.mult, op1=OP.add)


def build_block(nc: bass.Bass, bpc: int):
    tok = bpc * S
    # phase-2 chunks: 512-token chunks, tail merged into the last one
    nch = max(1, tok // 512)
    chunks = [(c0, 512) for c0 in range(0, (nch - 1) * 512, 512)]
    chunks.append(((nch - 1) * 512, tok - (nch - 1) * 512))

    x = nc.dram_tensor("x", [bpc, S, D], F32, kind="ExternalInput").ap().flatten_outer_dims()
    ln1_g = nc.dram_tensor("ln1_g", [D], F32, kind="ExternalInput").ap()
    ln1_b = nc.dram_tensor("ln1_b", [D], F32, kind="ExternalInput").ap()
    wq = nc.dram_tensor("Wq", [H, D, DH], F32, kind="ExternalInput").ap()
    bq = nc.dram_tensor("bq", [H, DH], F32, kind="ExternalInput").ap()
    wk = nc.dram_tensor("Wk", [H, D, DH], F32, kind="ExternalInput").ap()
    bk = nc.dram_tensor("bk", [H, DH], F32, kind="ExternalInput").ap()
    wv = nc.dram_tensor("Wv", [H, D, DH], F32, kind="ExternalInput").ap()
    bv = nc.dram_tensor("bv", [H, DH], F32, kind="ExternalInput").ap()
    wo = nc.dram_tensor("Wo", [D, D], F32, kind="ExternalInput").ap()
    bo = nc.dram_tensor("bo", [D], F32, kind="ExternalInput").ap()
    ln2_g = nc.dram_tensor("ln2_g", [D], F32, kind="ExternalInput").ap()
    ln2_b = nc.dram_tensor("ln2_b", [D], F32, kind="ExternalInput").ap()
    w1 = nc.dram_tensor("W1", [D, FF], F32, kind="ExternalInput").ap()
    b1 = nc.dram_tensor("b1", [FF], F32, kind="ExternalInput").ap()
    w2 = nc.dram_tensor("W2", [FF, D], F32, kind="ExternalInput").ap()
    b2 = nc.dram_tensor("b2", [D], F32, kind="ExternalInput").ap()
    out = nc.dram_tensor("out", [bpc, S, D], F32, kind="ExternalOutput").ap().flatten_outer_dims()
    x2s = nc.dram_tensor("x2_scratch", [tok, D], F32, kind="Internal").ap()

    w8 = FP8 if USE_FP8 else BF16
    sq, so = (WS_QKV, WS_O) if USE_FP8 else (1.0, 1.0)

    with tile.TileContext(nc) as tc:
        import contextlib
        with contextlib.ExitStack() as res:
            # ---------------- resident constants ----------------
            singles = res.enter_context(tc.tile_pool(name="singles", bufs=1))
            small = res.enter_context(tc.tile_pool(name="small", bufs=4))

            ident = singles.tile([128, 128], BF16, name="ident")
            make_identity(nc, ident)
            ones_row = singles.tile([1, 128], BF16, name="ones_row")
            nc.vector.memset(ones_row, 1.0)
            ones_s = singles.tile([1, S], BF16, name="ones_s")
            nc.vector.memset(ones_s, 1.0)

            # bias rows for K=1 fold-in matmuls
            def load_row_bf16(src_ap, name, scale=1.0):
                row = bass.AP(tensor=src_ap.tensor, offset=src_ap.offset,
                              ap=[[0, 1]] + list(src_ap.ap))
                st = singles.tile([1, D], F32, tag="rowstage", name="rowst")
                nc.sync.dma_start(st, row)
                t = singles.tile([1, D], BF16, name=name)
                nc.vector.tensor_scalar_mul(t, st, scale)
                return t

            bo_row = load_row_bf16(bo, "bo_row")
            b2_row = load_row_bf16(b2, "b2_row", 64.0)
            bv_row = load_row_bf16(bv.rearrange("h e -> (h e)"), "bv_row", sq)

            # per-partition biases / LN gain+bias in feature-major layout
            bq_pp = singles.tile([128, NHP], F32, name="bq_pp")
            nc.gpsimd.dma_start(bq_pp, bq.rearrange("(hp two) e -> (two e) hp", two=2))
            bk_pp = singles.tile([128, NHP], F32, name="bk_pp")
            nc.gpsimd.dma_start(bk_pp, bk.rearrange("(hp two) e -> (two e) hp", two=2))
            b1_pp = singles.tile([128, MFF], F32, name="b1_pp")
            nc.gpsimd.dma_start(b1_pp, b1.rearrange("(m p) -> p m", p=128))
            ln_pps = {}
            for nm, src in (("ln1g", ln1_g), ("ln1b", ln1_b),
                            ("ln2g", ln2_g), ("ln2b", ln2_b)):
                t = singles.tile([128, KK], F32, name=f"{nm}_pp")
                nc.gpsimd.dma_start(t, src.rearrange("(kk p) -> p kk", p=128))
                ln_pps[nm] = t

            # ================= phase 1: attention (per batch) =================
            with contextlib.ExitStack() as p1:
                stage = p1.enter_context(tc.tile_pool(name="stage1", bufs=2))
                wpool = p1.enter_context(tc.tile_pool(name="wpool1", bufs=1))

                # --- attention weights, fp8 DoubleRow layout [128, G, 2, ...] ---
                wq_sbs = [wpool.tile([128, G, 2, 128], w8, name=f"wq_sb{hp}")
                          for hp in range(NHP)]
                wk_sbs = [wpool.tile([128, G, 2, 128], w8, name=f"wk_sb{hp}")
                          for hp in range(NHP)]
                wv_sb = wpool.tile([128, G, 2, D], w8, name="wv_sb")
                wo_sb = wpool.tile([128, G, 2, D], w8, name="wo_sb")

                def emit_weights():
                    for h in range(H):
                        for dsts, wsrc in ((wq_sbs, wq), (wk_sbs, wk)):
                            hp, par = h // 2, h % 2
                            st = stage.tile([128, G, 2, DH], F32, tag="stage",
                                            name="wqk_st")
                            nc.sync.dma_start(
                                st, wsrc[h].rearrange("(g ko p) e -> p g ko e",
                                                      g=G, ko=2))
                            nc.vector.tensor_scalar_mul(
                                dsts[hp][:, :, :, DH * par:DH * par + DH], st, sq)
                    for h in range(H):
                        st = stage.tile([128, G, 2, DH], F32, tag="stage", name="wv_st")
                        nc.sync.dma_start(
                            st, wv[h].rearrange("(g ko p) e -> p g ko e", g=G, ko=2))
                        nc.vector.tensor_scalar_mul(
                            wv_sb[:, :, :, DH * h:DH * h + DH], st, sq)
                    for g in range(G):
                        for ko in range(2):
                            st = stage.tile([128, D], F32, tag="stage", name="wo_st")
                            kk = 2 * g + ko
                            nc.sync.dma_start(st, wo[128 * kk:128 * (kk + 1), :])
                            nc.vector.tensor_scalar_mul(wo_sb[:, g, ko, :], st, so)

                # PSUM pools: see module docstring
                pmm = p1.enter_context(tc.tile_pool(name="pmm", bufs=2, space="PSUM"))
                psc = p1.enter_context(tc.tile_pool(name="psc", bufs=2, space="PSUM"))
                pav = p1.enter_context(tc.tile_pool(name="pav", bufs=1, space="PSUM"))

                xpool = p1.enter_context(tc.tile_pool(name="xpool", bufs=2))
                x2pool = p1.enter_context(tc.tile_pool(name="x2pool", bufs=2))
                hnpool = p1.enter_context(tc.tile_pool(name="hnpool", bufs=3))
                h1pool = p1.enter_context(tc.tile_pool(name="h1pool", bufs=2))
                qkpool = p1.enter_context(tc.tile_pool(name="qkpool", bufs=2))
                vpool = p1.enter_context(tc.tile_pool(name="vpool", bufs=2))
                atpool = p1.enter_context(tc.tile_pool(name="atpool", bufs=2))
                epool = p1.enter_context(tc.tile_pool(name="epool", bufs=2))
                aupool = p1.enter_context(tc.tile_pool(name="aupool", bufs=4))

                def qkv_mm(ps, w_ap3, x_ap3, first_extra=None):
                    """Accumulate over G DoubleRow groups (or 2G bf16 k-tiles)."""
                    last = first_extra is None
                    if USE_FP8:
                        for g in range(G):
                            nc.tensor.matmul(ps, w_ap3(g), x_ap3(g),
                                             start=(g == 0),
                                             stop=(last and g == G - 1),
                                             perf_mode=DR)
                    else:
                        for g in range(G):
                            for ko in range(2):
                                nc.tensor.matmul(ps, w_ap3(g, ko), x_ap3(g, ko),
                                                 start=(g == 0 and ko == 0),
                                                 stop=(last and g == G - 1 and ko == 1))
                    if first_extra is not None:
                        first_extra()

                def emit_ab(b):
                    base = b * S
                    # ---- A: load x, LN1, transpose to feature-major fp8 ----
                    x_sb = xpool.tile([128, len(S_TILES), D], F32, name="x_sb")
                    h1T = h1pool.tile([128, KK, SP], w8, name="h1T")
                    mvb = small.tile([128, len(S_TILES), 2], F32, tag="mvb", name="mvb")
                    nc.vector.memset(mvb, 1.0)
                    for i, (t0, rows) in enumerate(S_TILES):
                        nc.sync.dma_start(x_sb[:rows, i, :], x[base + t0: base + t0 + rows, :])
                        _ln_stats_tile(nc, small, x_sb[:rows, i, :], rows, mvb, i)
                    rstd_b = _rsqrt_batch(nc, small, mvb, len(S_TILES))
                    for i, (t0, rows) in enumerate(S_TILES):
                        hn = hnpool.tile([128, D], BF16, tag="hn", name="hn")
                        _ln_apply(nc, x_sb[:rows, i, :], rows, mvb, rstd_b, i, hn[:rows])
                        _transpose_to(nc, pmm, ident, hn, rows, h1T, t0,
                                      ln_pps["ln1g"], ln_pps["ln1b"], S)

                    # ---- B: QKV (fp8 DoubleRow) ----
                    q_sb = qkpool.tile([128, NHP, S], BF16, name="q_sb")
                    k_sb = qkpool.tile([128, NHP, S], BF16, name="k_sb")
                    for hp in range(NHP):
                        for which, (dst, wsb, bpp) in enumerate(
                                ((q_sb, wq_sbs, bq_pp), (k_sb, wk_sbs, bk_pp))):
                            for n0, nw in S_CHUNKS:
                                ps = pmm.tile([128, 512], F32, tag="mm1", name="qk_ps")
                                qkv_mm(
                                    ps[:, 0:nw],
                                    (lambda g, ko=None, n0=n0, nw=nw:
                                     wsb[hp][:, g, :, :] if ko is None
                                     else wsb[hp][:, g, ko, :]),
                                    (lambda g, ko=None, n0=n0, nw=nw:
                                     h1T[:, 2 * g:2 * g + 2, n0:n0 + nw] if ko is None
                                     else h1T[:, 2 * g + ko, n0:n0 + nw]))
                                if which == 0:
                                    nc.scalar.activation(
                                        dst[:, hp, n0:n0 + nw], ps[:, 0:nw],
                                        AF.Identity, bias=bpp[:, hp:hp + 1],
                                        scale=1.0 / sq)
                                else:
                                    nc.vector.tensor_scalar(
                                        dst[:, hp, n0:n0 + nw], ps[:, 0:nw],
                                        1.0 / sq, bpp[:, hp:hp + 1],
                                        op0=OP.mult, op1=OP.add)
                    # v kept at the x{sq} weight scale; the ones column carries
                    # sq*WS_O so the softmax normalization descales v AND Wo.
                    v_aug = vpool.tile([128, len(S_TILES), H, DH + 1], BF16, name="v_aug")
                    for i, (t0, rows) in enumerate(S_TILES):
                        for n0, nw in D_CHUNKS:
                            ps = pmm.tile([128, 512], F32, tag="mm1", name="v_ps")
                            qkv_mm(
                                ps[:rows, 0:nw],
                                (lambda g, ko=None, n0=n0, nw=nw:
                                 h1T[:, 2 * g:2 * g + 2, t0:t0 + rows] if ko is None
                                 else h1T[:, 2 * g + ko, t0:t0 + rows]),
                                (lambda g, ko=None, n0=n0, nw=nw:
                                 wv_sb[:, g, :, n0:n0 + nw] if ko is None
                                 else wv_sb[:, g, ko, n0:n0 + nw]),
                                first_extra=(lambda ps=ps, rows=rows, n0=n0, nw=nw:
                                             nc.tensor.matmul(
                                                 ps[:rows, 0:nw],
                                                 ones_row[0:1, 0:rows],
                                                 bv_row[0:1, n0:n0 + nw],
                                                 start=False, stop=True)))
                            nc.vector.tensor_copy(
                                v_aug[:rows, i, n0 // DH:(n0 + nw) // DH, 0:DH],
                                ps[:rows, 0:nw].rearrange("p (h e) -> p h e", e=DH))
                        nc.vector.memset(v_aug[:rows, i, :, DH:DH + 1], sq * so)

                    return dict(x_sb=x_sb, h1T=h1T, q_sb=q_sb, k_sb=k_sb,
                                v_aug=v_aug)

                def emit_cd(b, st):
                    base = b * S
                    x_sb, q_sb, k_sb, v_aug = (st["x_sb"], st["q_sb"],
                                               st["k_sb"], st["v_aug"])
                    # ---- C: attention per head pair ----
                    attnT = atpool.tile([128, KK, SP], w8, name="attnT")
                    for hp in range(NHP):
                        expT = epool.tile([128, len(S_TILES), 2, S], BF16,
                                          tag="expT", name="expT")
                        for j, (t0, rj) in enumerate(S_TILES):
                            sps_t = {}
                            # even/odd scores adjacent -> concurrent row-groups
                            for par in range(2):
                                off = DH * par
                                sps = psc.tile([128, S], F32, tag="sps", name="sc_ps")
                                sps_t[par] = sps
                                for n0, nw in S_CHUNKS:
                                    nc.tensor.matmul(sps[:rj, n0:n0 + nw],
                                                     k_sb[off:off + DH, hp, t0:t0 + rj],
                                                     q_sb[off:off + DH, hp, n0:n0 + nw],
                                                     start=True, stop=True)
                            for par in range(2):
                                nc.scalar.activation(expT[:rj, j, par, :],
                                                     sps_t[par][:rj, :],
                                                     AF.Exp, bias=0.0, scale=SCALE)
                        for par in range(2):
                            h = 2 * hp + par
                            off = DH * par
                            aps = pav.tile([DH + 1, S], F32, tag="aps", name="attn_ps")
                            for n0, nw in S_CHUNKS:
                                for j, (t0, rj) in enumerate(S_TILES):
                                    nc.tensor.matmul(aps[:, n0:n0 + nw],
                                                     v_aug[:rj, j, h, :],
                                                     expT[:rj, j, par, n0:n0 + nw],
                                                     start=(j == 0),
                                                     stop=(j == len(S_TILES) - 1))
                            # evacuate PSUM immediately (ACT) so the next head's
                            # PV can start; normalize from SBUF off the chain
                            att_un = aupool.tile([DH + 1, S], BF16, tag="attun",
                                                 name="att_un")
                            nc.scalar.activation(att_un, aps, AF.Identity)
                            rec = small.tile([1, S], BF16, tag="rec", name="rec")
                            with nc.allow_low_precision(reason="softmax denom bf16"):
                                nc.vector.reciprocal(rec, att_un[DH:DH + 1, :])
                            rec_bc = small.tile([DH, S], BF16, tag="recbc", name="rec_bc")
                            nc.gpsimd.partition_broadcast(rec_bc, rec, channels=DH)
                            nc.vector.tensor_tensor(attnT[off:off + DH, hp, 0:S],
                                                    att_un[0:DH, :], rec_bc, OP.mult)

                    # ---- D: out-proj (fp8 DoubleRow) + residual -> DRAM ----
                    x2t = x2pool.tile([128, len(S_TILES), D], F32, name="x2t")
                    for i, (t0, rows) in enumerate(S_TILES):
                        for n0, nw in D_CHUNKS:
                            ops = pmm.tile([128, 512], F32, tag="mm1", name="op_ps")
                            qkv_mm(
                                ops[:rows, 0:nw],
                                (lambda g, ko=None, t0=t0, rows=rows:
                                 attnT[:, 2 * g:2 * g + 2, t0:t0 + rows] if ko is None
                                 else attnT[:, 2 * g + ko, t0:t0 + rows]),
                                (lambda g, ko=None, n0=n0, nw=nw:
                                 wo_sb[:, g, :, n0:n0 + nw] if ko is None
                                 else wo_sb[:, g, ko, n0:n0 + nw]),
                                first_extra=(lambda ops=ops, rows=rows, n0=n0, nw=nw:
                                             nc.tensor.matmul(
                                                 ops[:rows, 0:nw],
                                                 ones_row[0:1, 0:rows],
                                                 bo_row[0:1, n0:n0 + nw],
                                                 start=False, stop=True)))
                            nc.vector.tensor_tensor(x2t[:rows, i, n0:n0 + nw],
                                                    ops[:rows, 0:nw],
                                                    x_sb[:rows, i, n0:n0 + nw], OP.add)
                        nc.sync.dma_start(x2s[base + t0: base + t0 + rows, :],
                                          x2t[:rows, i, :])

                # software-pipelined emission: batch b+1's loads/LN/QKV are
                # emitted before batch b's attention so the scheduler always
                # has independent PE work during the ACT-bound softmax.
                emit_weights()
                prev = None
                for b in range(bpc):
                    st = emit_ab(b)
                    if prev is not None:
                        emit_cd(b - 1, prev)
                    prev = st
                emit_cd(bpc - 1, prev)

            # ================= phase 2: MLP (bf16, per chunk) =================
            with contextlib.ExitStack() as p2:
                pf1 = p2.enter_context(tc.tile_pool(name="pf1", bufs=3, space="PSUM"))
                pf2 = p2.enter_context(tc.tile_pool(name="pf2", bufs=3, space="PSUM"))
                ptp = p2.enter_context(tc.tile_pool(name="ptp2", bufs=2, space="PSUM"))
                stage = p2.enter_context(tc.tile_pool(name="stage2", bufs=2))
                w1pool = p2.enter_context(tc.tile_pool(name="w1pool", bufs=1))
                w2pool = p2.enter_context(tc.tile_pool(name="w2pool", bufs=1))
                x2cpool = p2.enter_context(tc.tile_pool(name="x2cpool", bufs=2))
                h2pool = p2.enter_context(tc.tile_pool(name="h2pool", bufs=2))
                hnpool = p2.enter_context(tc.tile_pool(name="hnpool2", bufs=2))
                mpool = p2.enter_context(tc.tile_pool(name="mpool", bufs=1))
                opool = p2.enter_context(tc.tile_pool(name="opool", bufs=2))

                cmax = max(cw for _, cw in chunks)
                ntile_max = (cmax + 127) // 128

                w1_sb = w1pool.tile([128, KK, MFF, 128], BF16, name="w1_sb")
                for kk in range(KK):
                    for half in range(2):
                        st = stage.tile([128, FF // 2], F32, tag="stage", name="w1_st")
                        nc.sync.dma_start(
                            st, w1[128 * kk:128 * (kk + 1),
                                   (FF // 2) * half:(FF // 2) * (half + 1)])
                        nc.vector.tensor_copy(
                            w1_sb[:, kk, 12 * half:12 * (half + 1), :]
                            .rearrange("p m e -> p (m e)"), st)
                # fc2 contraction split: m-tiles 0-11 run fp8-e4m3 DoubleRow,
                # 12-23 bf16; BOTH halves' weights are pre-scaled x64
                # (lossless for bf16) so they share one PSUM group, and the
                # 1/64 descale fuses into the residual-add epilogue.
                # Validated rel 0.0163 total vs 2e-2 budget.
                MF8 = 12 if USE_FP8 else 0
                G2 = MF8 // 2
                CP = (cmax + 15) // 16 * 16
                if USE_FP8:
                    w2_8 = w2pool.tile([128, G2, 2, D], FP8, name="w2_8")
                w2_sb = w2pool.tile([128, MFF - MF8, D], BF16, name="w2_sb")
                for m in range(MFF):
                    st = stage.tile([128, D], F32, tag="stage", name="w2_st")
                    nc.sync.dma_start(st, w2[128 * m:128 * (m + 1), :])
                    if m < MF8:
                        nc.vector.tensor_scalar_mul(w2_8[:, m // 2, m % 2, :],
                                                    st, 64.0)
                    else:
                        nc.vector.tensor_scalar_mul(w2_sb[:, m - MF8, :], st, 64.0)

                def emit_ln2(c0, cw):
                    ctiles = [(i0, min(128, cw - i0)) for i0 in range(0, cw, 128)]
                    x2c = x2cpool.tile([128, ntile_max, D], F32, name="x2c")
                    h2T = h2pool.tile([128, KK, cmax], BF16, name="h2T")
                    mvb = small.tile([128, ntile_max, 2], F32, tag="mvb", name="mvb2")
                    nc.vector.memset(mvb, 1.0)
                    for i, (i0, rows) in enumerate(ctiles):
                        nc.sync.dma_start(x2c[:rows, i, :],
                                          x2s[c0 + i0: c0 + i0 + rows, :])
                        _ln_stats_tile(nc, small, x2c[:rows, i, :], rows, mvb, i)
                    rstd_b = _rsqrt_batch(nc, small, mvb, len(ctiles))
                    for i, (i0, rows) in enumerate(ctiles):
                        hn = hnpool.tile([128, D], BF16, tag="hn", name="hn2")
                        _ln_apply(nc, x2c[:rows, i, :], rows, mvb, rstd_b, i, hn[:rows])
                        _transpose_to(nc, ptp, ident, hn, rows, h2T, i0,
                                      ln_pps["ln2g"], ln_pps["ln2b"], cw)
                    return x2c, h2T

                def emit_mlp(c0, cw, x2c, h2T):
                    ctiles = [(i0, min(128, cw - i0)) for i0 in range(0, cw, 128)]
                    cchunks = [(n0, min(512, cw - n0)) for n0 in range(0, cw, 512)]
                    if USE_FP8:
                        m_sb8 = mpool.tile([128, G2, 2, CP], FP8, tag="m8",
                                           name="m_sb8")
                    m_sb = mpool.tile([128, MFF - MF8, cmax], BF16, name="m_sb")
                    for m in range(MFF):
                        for n0, nw in cchunks:
                            fps = pf1.tile([128, 512], F32, tag="f1", name="fc1_ps")
                            for kk in range(KK):
                                nc.tensor.matmul(fps[:, 0:nw], w1_sb[:, kk, m, :],
                                                 h2T[:, kk, n0:n0 + nw],
                                                 start=(kk == 0), stop=(kk == KK - 1))
                            gdst = (m_sb8[:, m // 2, m % 2, n0:n0 + nw] if m < MF8
                                    else m_sb[:, m - MF8, n0:n0 + nw])
                            nc.scalar.activation(gdst, fps[:, 0:nw],
                                                 AF.Gelu_apprx_tanh,
                                                 bias=b1_pp[:, m:m + 1], scale=1.0)
                    for i, (i0, rows) in enumerate(ctiles):
                        ot = opool.tile([128, D], F32, tag="ot", name="ot")
                        for n0, nw in D_CHUNKS:
                            gps = pf2.tile([128, 512], F32, tag="f2", name="fc2_ps")
                            for g in range(G2):
                                nc.tensor.matmul(gps[:rows, 0:nw],
                                                 m_sb8[:, g, :, i0:i0 + rows],
                                                 w2_8[:, g, :, n0:n0 + nw],
                                                 start=(g == 0), stop=False,
                                                 perf_mode=DR)
                            for m in range(MFF - MF8):
                                nc.tensor.matmul(gps[:rows, 0:nw],
                                                 m_sb[:, m, i0:i0 + rows],
                                                 w2_sb[:, m, n0:n0 + nw],
                                                 start=(G2 == 0 and m == 0),
                                                 stop=False)
                            nc.tensor.matmul(gps[:rows, 0:nw],
                                             ones_row[0:1, 0:rows],
                                             b2_row[0:1, n0:n0 + nw],
                                             start=False, stop=True)
                            nc.vector.scalar_tensor_tensor(
                                ot[:rows, n0:n0 + nw], gps[:rows, 0:nw],
                                1.0 / 64.0, x2c[:rows, i, n0:n0 + nw],
                                OP.mult, OP.add)
                        nc.sync.dma_start(out[c0 + i0: c0 + i0 + rows, :], ot[:rows, :])

                prevc = None
                for c0, cw in chunks:
                    st2 = emit_ln2(c0, cw)
                    if prevc is not None:
                        emit_mlp(*prevc)
                    prevc = (c0, cw, *st2)
                emit_mlp(*prevc)
    return nc


_NC_CACHE = {}


def build_nc(bpc=B // NCORES):
    if bpc not in _NC_CACHE:
        from concourse import bacc
        nc = bacc.Bacc("TRN2", target_bir_lowering=False, debug=False)
        build_block(nc, bpc)
        nc.compile()
        _NC_CACHE[bpc] = nc
    return _NC_CACHE[bpc]


def run(inputs, **spmd_kwargs):
    from concourse.bass_utils import run_bass_kernel_spmd

    inputs = {k: np.ascontiguousarray(np.asarray(v, dtype=np.float32))
              for k, v in inputs.items()}
    x_full = inputs["x"]
    bpc = B // NCORES
    nc = build_nc(bpc)
    weights = {k: v for k, v in inputs.items() if k != "x"}
    in_maps = [dict(weights, x=np.ascontiguousarray(x_full[c * bpc:(c + 1) * bpc]))
               for c in range(NCORES)]
    res = run_bass_kernel_spmd(nc, in_maps, core_ids=list(range(NCORES)),
                               **spmd_kwargs)
    out = np.concatenate([r["out"] for r in res.results], axis=0)
    return out, res


def kernel(**inputs):
    return run(inputs)[0]


# revision 54
# speedup vs baseline: 1.0234x; 1.0234x over previous
"""Trainium2 Bass kernel: ViT-style dense transformer block (B=64,S=577,D=768,H=12).

Sharding: pure data-parallel over batch across 8 NeuronCores (8 batches/core,
no collectives).  Per core:

  Phase 1 (per batch): LN1 -> QKV -> attention -> out-proj + residual,
    spilling the residual stream x2 to DRAM scratch.
    - QKV and the out-projection run in fp8-e4m3 with DoubleRow perf mode
      (2 contraction rows / cycle).  Weights are pre-scaled (x16 for
      Wq/Wk/Wv with a 1/16 descale folded into the PSUM->SBUF epilogue;
      x4 for Wo with the 1/4 descale folded into the softmax denominator
      via a 4.0-valued ones column in V).
    - Scores are computed transposed (scoresT[j,i] = k_j . q_i) per head
      with K=64; the even/odd heads of a pair are issued back-to-back so
      they run concurrently in disjoint PE row-groups.
    - Softmax denominator comes from the extra 4.0 column appended to V;
      normalization = reciprocal + gpsimd partition-broadcast + multiply.
    - Biases bo (and bv) are folded into the matmul accumulation as K=1
      ones-row matmuls; bq/bk ride the PSUM->SBUF epilogues.

  Phase 2 (per 512-token chunk, last chunk 520): LN2 -> fc1 + tanh-GELU
    (scalar engine) -> fc2 + residual, all bf16 (fp8 fails the accuracy
    budget for the MLP), b2 folded into the fc2 accumulation.

PSUM plan (8 banks), phase 1: scores ring2 x 2 banks, PV accumulator
ring1 x 2 banks, and a shared ring2 of 1-bank tiles for QKV / out-proj
chunks + PE transposes — so next-batch QKV always has PSUM available
while the current batch's ACT-bound softmax runs.
"""

import math
import numpy as np

import concourse.bass as bass
import concourse.mybir as mybir
import concourse.tile as tile
from concourse.masks import make_identity

F32 = mybir.dt.float32
I32 = mybir.dt.int32
BF16 = mybir.dt.bfloat16
FP8 = mybir.dt.float8e4
AF = mybir.ActivationFunctionType
OP = mybir.AluOpType
DR = mybir.MatmulPerfMode.DoubleRow
RSQRT_MAGIC = 0x5F3759DF

B, S, D, H, DH = 64, 577, 768, 12, 64
SP = 592               # S padded so fp8 DoubleRow ko-steps are 16B-aligned
FF = 4 * D
EPS = 1e-6
NCORES = 8
KK = D // 128          # 6 k-tiles over D
G = KK // 2            # 3 DoubleRow k-groups
MFF = FF // 128        # 24 tiles over FF
NHP = H // 2           # 6 head pairs
SCALE = 1.0 / math.sqrt(DH)
WS_QKV = 16.0          # fp8 weight pre-scale for Wq/Wk/Wv
WS_O = 4.0             # fp8 weight pre-scale for Wo (descale via V ones col)

USE_FP8 = True

# token tiles within one sequence: 4 x 128 + 65
S_TILES = [(i * 128, min(128, S - i * 128)) for i in range((S + 127) // 128)]
# n-chunks over S and D for PSUM-bank-sized matmul outputs
S_CHUNKS = [(0, 512), (512, S - 512)]
D_CHUNKS = [(0, 512), (512, D - 512)]


def _bcast(ap):
    """[N] dram AP -> [128, N] partition-broadcast AP."""
    return bass.AP(tensor=ap.tensor, offset=ap.offset, ap=[[0, 128]] + list(ap.ap))


def _ln_stats_tile(nc, pool, x_sl, rows, mvb, i):
    """bn stats over the free dim (768) of x_sl[:rows] -> mvb[:, i, :]=(mu,var)."""
    stats = pool.tile([128, 3, 6], F32, tag="lnstats", name="lnstats")
    for sg in range(3):
        nc.vector.bn_stats(stats[:rows, sg, :], x_sl[:, 256 * sg:256 * (sg + 1)])
    nc.vector.bn_aggr(mvb[:rows, i, :], stats[:rows])


def _rsqrt_batch(nc, pool, mvb, n):
    """rstd[:, i] = 1/sqrt(var_i + EPS), magic-constant + 2 Newton iters on DVE."""
    veps = pool.tile([128, 8], F32, tag="lnveps", name="veps")
    nc.vector.tensor_scalar_add(veps[:, :n], mvb[:, 0:n, 1], EPS)
    hv = pool.tile([128, 8], F32, tag="lnhv", name="hv")
    nc.vector.tensor_scalar_mul(hv[:, :n], veps[:, :n], 0.5)
    y = pool.tile([128, 8], F32, tag="lnrstd", name="rstd_b")
    t = pool.tile([128, 8], F32, tag="lnnt", name="nt")
    nc.vector.tensor_scalar(t[:, :n].bitcast(I32), veps[:, :n].bitcast(I32),
                            1, None, op0=OP.arith_shift_right)
    nc.vector.tensor_scalar(y[:, :n].bitcast(I32), t[:, :n].bitcast(I32),
                            -1, RSQRT_MAGIC, op0=OP.mult, op1=OP.add)
    for _ in range(2):
        nc.vector.tensor_tensor(t[:, :n], y[:, :n], y[:, :n], OP.mult)
        nc.vector.tensor_tensor(t[:, :n], t[:, :n], hv[:, :n], OP.mult)
        nc.vector.tensor_scalar(t[:, :n], t[:, :n], -1.0, 1.5,
                                op0=OP.mult, op1=OP.add)
        nc.vector.tensor_tensor(y[:, :n], y[:, :n], t[:, :n], OP.mult)
    return y


def _ln_apply(nc, x_sl, rows, mvb, rstd_b, i, out_sl):
    """(x - mu_i) * rstd_i -> out_sl (one DVE op, per-partition scalars)."""
    nc.vector.tensor_scalar(out_sl, x_sl, mvb[:rows, i, 0:1],
                            rstd_b[:rows, i:i + 1],
                            op0=OP.subtract, op1=OP.mult)


def _transpose_to(nc, ptp, ident, src, rows, dst, col0, g_pp, b_pp, ncols):
    """src[:rows, 0:768] bf16 -> dst[:, kk, col0:col0+rows] feature-major,
    applying the LN gain/bias as per-partition scalars during the DVE copy."""
    for kk in range(KK):
        tp = ptp.tile([128, 128], BF16, tag="mm1", name="tp")
        nc.tensor.transpose(tp[:, :rows], src[:rows, 128 * kk:128 * (kk + 1)],
                            ident[:rows, :rows])
        nc.vector.tensor_scalar(dst[:, kk, col0:col0 + rows], tp[:, :rows],
                                g_pp[:, kk:kk + 1], b_pp[:, kk:kk + 1],
                                op0=OP.mult, op1=OP.add)


def build_block(nc: bass.Bass, bpc: int):
    tok = bpc * S
    # phase-2 chunks: 512-token chunks, tail merged into the last one
    nch = max(1, tok // 512)
    chunks = [(c0, 512) for c0 in range(0, (nch - 1) * 512, 512)]
    chunks.append(((nch - 1) * 512, tok - (nch - 1) * 512))

    x = nc.dram_tensor("x", [bpc, S, D], F32, kind="ExternalInput").ap().flatten_outer_dims()
    ln1_g = nc.dram_tensor("ln1_g", [D], F32, kind="ExternalInput").ap()
    ln1_b = nc.dram_tensor("ln1_b", [D], F32, kind="ExternalInput").ap()
    wq = nc.dram_tensor("Wq", [H, D, DH], F32, kind="ExternalInput").ap()
    bq = nc.dram_tensor("bq", [H, DH], F32, kind="ExternalInput").ap()
    wk = nc.dram_tensor("Wk", [H, D, DH], F32, kind="ExternalInput").ap()
    bk = nc.dram_tensor("bk", [H, DH], F32, kind="ExternalInput").ap()
    wv = nc.dram_tensor("Wv", [H, D, DH], F32, kind="ExternalInput").ap()
    bv = nc.dram_tensor("bv", [H, DH], F32, kind="ExternalInput").ap()
    wo = nc.dram_tensor("Wo", [D, D], F32, kind="ExternalInput").ap()
    bo = nc.dram_tensor("bo", [D], F32, kind="ExternalInput").ap()
    ln2_g = nc.dram_tensor("ln2_g", [D], F32, kind="ExternalInput").ap()
    ln2_b = nc.dram_tensor("ln2_b", [D], F32, kind="ExternalInput").ap()
    w1 = nc.dram_tensor("W1", [D, FF], F32, kind="ExternalInput").ap()
    b1 = nc.dram_tensor("b1", [FF], F32, kind="ExternalInput").ap()
    w2 = nc.dram_tensor("W2", [FF, D], F32, kind="ExternalInput").ap()
    b2 = nc.dram_tensor("b2", [D], F32, kind="ExternalInput").ap()
    out = nc.dram_tensor("out", [bpc, S, D], F32, kind="ExternalOutput").ap().flatten_outer_dims()
    x2s = nc.dram_tensor("x2_scratch", [tok, D], F32, kind="Internal").ap()

    w8 = FP8 if USE_FP8 else BF16
    sq, so = (WS_QKV, WS_O) if USE_FP8 else (1.0, 1.0)

    with tile.TileContext(nc) as tc:
        import contextlib
        with contextlib.ExitStack() as res:
            # ---------------- resident constants ----------------
            singles = res.enter_context(tc.tile_pool(name="singles", bufs=1))
            small = res.enter_context(tc.tile_pool(name="small", bufs=4))

            ident = singles.tile([128, 128], BF16, name="ident")
            make_identity(nc, ident)
            ones_row = singles.tile([1, 128], BF16, name="ones_row")
            nc.vector.memset(ones_row, 1.0)
            ones_s = singles.tile([1, S], BF16, name="ones_s")
            nc.vector.memset(ones_s, 1.0)

            # bias rows for K=1 fold-in matmuls
            def load_row_bf16(src_ap, name, scale=1.0):
                row = bass.AP(tensor=src_ap.tensor, offset=src_ap.offset,
                              ap=[[0, 1]] + list(src_ap.ap))
                st = singles.tile([1, D], F32, tag="rowstage", name="rowst")
                nc.sync.dma_start(st, row)
                t = singles.tile([1, D], BF16, name=name)
                nc.vector.tensor_scalar_mul(t, st, scale)
                return t

            bo_row = load_row_bf16(bo, "bo_row")
            b2_row = load_row_bf16(b2, "b2_row", 64.0)
            bv_row = load_row_bf16(bv.rearrange("h e -> (h e)"), "bv_row", sq)

            # per-partition biases / LN gain+bias in feature-major layout
            bq_pp = singles.tile([128, NHP], F32, name="bq_pp")
            nc.gpsimd.dma_start(bq_pp, bq.rearrange("(hp two) e -> (two e) hp", two=2))
            bk_pp = singles.tile([128, NHP], F32, name="bk_pp")
            nc.gpsimd.dma_start(bk_pp, bk.rearrange("(hp two) e -> (two e) hp", two=2))
            b1_pp = singles.tile([128, MFF], F32, name="b1_pp")
            nc.gpsimd.dma_start(b1_pp, b1.rearrange("(m p) -> p m", p=128))
            ln_pps = {}
            for nm, src in (("ln1g", ln1_g), ("ln1b", ln1_b),
                            ("ln2g", ln2_g), ("ln2b", ln2_b)):
                t = singles.tile([128, KK], F32, name=f"{nm}_pp")
                nc.gpsimd.dma_start(t, src.rearrange("(kk p) -> p kk", p=128))
                ln_pps[nm] = t

            # ================= phase 1: attention (per batch) =================
            with contextlib.ExitStack() as p1:
                stage = p1.enter_context(tc.tile_pool(name="stage1", bufs=2))
                wpool = p1.enter_context(tc.tile_pool(name="wpool1", bufs=1))

                # --- attention weights, fp8 DoubleRow layout [128, G, 2, ...] ---
                wq_sbs = [wpool.tile([128, G, 2, 128], w8, name=f"wq_sb{hp}")
                          for hp in range(NHP)]
                wk_sbs = [wpool.tile([128, G, 2, 128], w8, name=f"wk_sb{hp}")
                          for hp in range(NHP)]
                wv_sb = wpool.tile([128, G, 2, D], w8, name="wv_sb")
                wo_sb = wpool.tile([128, G, 2, D], w8, name="wo_sb")

                def emit_weights():
                    for h in range(H):
                        for dsts, wsrc in ((wq_sbs, wq), (wk_sbs, wk)):
                            hp, par = h // 2, h % 2
                            st = stage.tile([128, G, 2, DH], F32, tag="stage",
                                            name="wqk_st")
                            nc.sync.dma_start(
                                st, wsrc[h].rearrange("(g ko p) e -> p g ko e",
                                                      g=G, ko=2))
                            nc.vector.tensor_scalar_mul(
                                dsts[hp][:, :, :, DH * par:DH * par + DH], st, sq)
                    for h in range(H):
                        st = stage.tile([128, G, 2, DH], F32, tag="stage", name="wv_st")
                        nc.sync.dma_start(
                            st, wv[h].rearrange("(g ko p) e -> p g ko e", g=G, ko=2))
                        nc.vector.tensor_scalar_mul(
                            wv_sb[:, :, :, DH * h:DH * h + DH], st, sq)
                    for g in range(G):
                        for ko in range(2):
                            st = stage.tile([128, D], F32, tag="stage", name="wo_st")
                            kk = 2 * g + ko
                            nc.sync.dma_start(st, wo[128 * kk:128 * (kk + 1), :])
                            nc.vector.tensor_scalar_mul(wo_sb[:, g, ko, :], st, so)

                # PSUM pools: see module docstring
                pmm = p1.enter_context(tc.tile_pool(name="pmm", bufs=2, space="PSUM"))
                psc = p1.enter_context(tc.tile_pool(name="psc", bufs=2, space="PSUM"))
                pav = p1.enter_context(tc.tile_pool(name="pav", bufs=1, space="PSUM"))

                xpool = p1.enter_context(tc.tile_pool(name="xpool", bufs=2))
                x2pool = p1.enter_context(tc.tile_pool(name="x2pool", bufs=2))
                hnpool = p1.enter_context(tc.tile_pool(name="hnpool", bufs=3))
                h1pool = p1.enter_context(tc.tile_pool(name="h1pool", bufs=2))
                qkpool = p1.enter_context(tc.tile_pool(name="qkpool", bufs=2))
                vpool = p1.enter_context(tc.tile_pool(name="vpool", bufs=2))
                atpool = p1.enter_context(tc.tile_pool(name="atpool", bufs=2))
                epool = p1.enter_context(tc.tile_pool(name="epool", bufs=2))
                aupool = p1.enter_context(tc.tile_pool(name="aupool", bufs=4))

                def qkv_mm(ps, w_ap3, x_ap3, first_extra=None):
                    """Accumulate over G DoubleRow groups (or 2G bf16 k-tiles)."""
                    last = first_extra is None
                    if USE_FP8:
                        for g in range(G):
                            nc.tensor.matmul(ps, w_ap3(g), x_ap3(g),
                                             start=(g == 0),
                                             stop=(last and g == G - 1),
                                             perf_mode=DR)
                    else:
                        for g in range(G):
                            for ko in range(2):
                                nc.tensor.matmul(ps, w_ap3(g, ko), x_ap3(g, ko),
                                                 start=(g == 0 and ko == 0),
                                                 stop=(last and g == G - 1 and ko == 1))
                    if first_extra is not None:
                        first_extra()

                def emit_ab(b):
                    base = b * S
                    # ---- A: load x, LN1, transpose to feature-major fp8 ----
                    x_sb = xpool.tile([128, len(S_TILES), D], F32, name="x_sb")
                    h1T = h1pool.tile([128, KK, SP], w8, name="h1T")
                    mvb = small.tile([128, len(S_TILES), 2], F32, tag="mvb", name="mvb")
                    nc.vector.memset(mvb, 1.0)
                    for i, (t0, rows) in enumerate(S_TILES):
                        nc.sync.dma_start(x_sb[:rows, i, :], x[base + t0: base + t0 + rows, :])
                        _ln_stats_tile(nc, small, x_sb[:rows, i, :], rows, mvb, i)
                    rstd_b = _rsqrt_batch(nc, small, mvb, len(S_TILES))
                    for i, (t0, rows) in enumerate(S_TILES):
                        hn = hnpool.tile([128, D], BF16, tag="hn", name="hn")
                        _ln_apply(nc, x_sb[:rows, i, :], rows, mvb, rstd_b, i, hn[:rows])
                        _transpose_to(nc, pmm, ident, hn, rows, h1T, t0,
                                      ln_pps["ln1g"], ln_pps["ln1b"], S)

                    # ---- B: QKV (fp8 DoubleRow) ----
                    # per-hp q/k tiles: pair-0 attention starts as soon as its
                    # own epilogues land (helps the batch-0 pipeline fill)
                    q_sb = [qkpool.tile([128, S], BF16, name=f"q_sb{hp}")
                            for hp in range(NHP)]
                    k_sb = [qkpool.tile([128, S], BF16, name=f"k_sb{hp}")
                            for hp in range(NHP)]
                    for hp in range(NHP):
                        for which, (dst, wsb, bpp) in enumerate(
                                ((q_sb[hp], wq_sbs, bq_pp), (k_sb[hp], wk_sbs, bk_pp))):
                            for n0, nw in S_CHUNKS:
                                ps = pmm.tile([128, 512], F32, tag="mm1", name="qk_ps")
                                qkv_mm(
                                    ps[:, 0:nw],
                                    (lambda g, ko=None, n0=n0, nw=nw:
                                     wsb[hp][:, g, :, :] if ko is None
                                     else wsb[hp][:, g, ko, :]),
                                    (lambda g, ko=None, n0=n0, nw=nw:
                                     h1T[:, 2 * g:2 * g + 2, n0:n0 + nw] if ko is None
                                     else h1T[:, 2 * g + ko, n0:n0 + nw]))
                                if which == 0:
                                    nc.scalar.activation(
                                        dst[:, n0:n0 + nw], ps[:, 0:nw],
                                        AF.Identity, bias=bpp[:, hp:hp + 1],
                                        scale=1.0 / sq)
                                else:
                                    nc.vector.tensor_scalar(
                                        dst[:, n0:n0 + nw], ps[:, 0:nw],
                                        1.0 / sq, bpp[:, hp:hp + 1],
                                        op0=OP.mult, op1=OP.add)
                    # v kept at the x{sq} weight scale; the ones column carries
                    # sq*WS_O so the softmax normalization descales v AND Wo.
                    v_aug = vpool.tile([128, len(S_TILES), H, DH + 1], BF16, name="v_aug")
                    for i, (t0, rows) in enumerate(S_TILES):
                        for n0, nw in D_CHUNKS:
                            ps = pmm.tile([128, 512], F32, tag="mm1", name="v_ps")
                            qkv_mm(
                                ps[:rows, 0:nw],
                                (lambda g, ko=None, n0=n0, nw=nw:
                                 h1T[:, 2 * g:2 * g + 2, t0:t0 + rows] if ko is None
                                 else h1T[:, 2 * g + ko, t0:t0 + rows]),
                                (lambda g, ko=None, n0=n0, nw=nw:
                                 wv_sb[:, g, :, n0:n0 + nw] if ko is None
                                 else wv_sb[:, g, ko, n0:n0 + nw]),
                                first_extra=(lambda ps=ps, rows=rows, n0=n0, nw=nw:
                                             nc.tensor.matmul(
                                                 ps[:rows, 0:nw],
                                                 ones_row[0:1, 0:rows],
                                                 bv_row[0:1, n0:n0 + nw],
                                                 start=False, stop=True)))
                            nc.vector.tensor_copy(
                                v_aug[:rows, i, n0 // DH:(n0 + nw) // DH, 0:DH],
                                ps[:rows, 0:nw].rearrange("p (h e) -> p h e", e=DH))
                        nc.vector.memset(v_aug[:rows, i, :, DH:DH + 1], sq * so)

                    return dict(x_sb=x_sb, h1T=h1T, q_sb=q_sb, k_sb=k_sb,
                                v_aug=v_aug)

                def emit_cd(b, st):
                    base = b * S
                    x_sb, q_sb, k_sb, v_aug = (st["x_sb"], st["q_sb"],
                                               st["k_sb"], st["v_aug"])
                    # ---- C: attention per head pair ----
                    attnT = atpool.tile([128, KK, SP], w8, name="attnT")
                    for hp in range(NHP):
                        expT = epool.tile([128, len(S_TILES), 2, S], BF16,
                                          tag="expT", name="expT")
                        for j, (t0, rj) in enumerate(S_TILES):
                            sps_t = {}
                            # even/odd scores adjacent -> concurrent row-groups
                            for par in range(2):
                                off = DH * par
                                sps = psc.tile([128, S], F32, tag="sps", name="sc_ps")
                                sps_t[par] = sps
                                for n0, nw in S_CHUNKS:
                                    nc.tensor.matmul(sps[:rj, n0:n0 + nw],
                                                     k_sb[hp][off:off + DH, t0:t0 + rj],
                                                     q_sb[hp][off:off + DH, n0:n0 + nw],
                                                     start=True, stop=True)
                            for par in range(2):
                                nc.scalar.activation(expT[:rj, j, par, :],
                                                     sps_t[par][:rj, :],
                                                     AF.Exp, bias=0.0, scale=SCALE)
                        for par in range(2):
                            h = 2 * hp + par
                            off = DH * par
                            aps = pav.tile([DH + 1, S], F32, tag="aps", name="attn_ps")
                            for n0, nw in S_CHUNKS:
                                for j, (t0, rj) in enumerate(S_TILES):
                                    nc.tensor.matmul(aps[:, n0:n0 + nw],
                                                     v_aug[:rj, j, h, :],
                                                     expT[:rj, j, par, n0:n0 + nw],
                                                     start=(j == 0),
                                                     stop=(j == len(S_TILES) - 1))
                            # evacuate PSUM immediately (ACT) so the next head's
                            # PV can start; normalize from SBUF off the chain
                            att_un = aupool.tile([DH + 1, S], BF16, tag="attun",
                                                 name="att_un")
                            nc.scalar.activation(att_un, aps, AF.Identity)
                            rec = small.tile([1, S], BF16, tag="rec", name="rec")
                            with nc.allow_low_precision(reason="softmax denom bf16"):
                                nc.vector.reciprocal(rec, att_un[DH:DH + 1, :])
                            rec_bc = small.tile([DH, S], BF16, tag="recbc", name="rec_bc")
                            nc.gpsimd.partition_broadcast(rec_bc, rec, channels=DH)
                            nc.vector.tensor_tensor(attnT[off:off + DH, hp, 0:S],
                                                    att_un[0:DH, :], rec_bc, OP.mult)

                    # ---- D: out-proj (fp8 DoubleRow) + residual -> DRAM ----
                    x2t = x2pool.tile([128, len(S_TILES), D], F32, name="x2t")
                    for i, (t0, rows) in enumerate(S_TILES):
                        for n0, nw in D_CHUNKS:
                            ops = pmm.tile([128, 512], F32, tag="mm1", name="op_ps")
                            qkv_mm(
                                ops[:rows, 0:nw],
                                (lambda g, ko=None, t0=t0, rows=rows:
                                 attnT[:, 2 * g:2 * g + 2, t0:t0 + rows] if ko is None
                                 else attnT[:, 2 * g + ko, t0:t0 + rows]),
                                (lambda g, ko=None, n0=n0, nw=nw:
                                 wo_sb[:, g, :, n0:n0 + nw] if ko is None
                                 else wo_sb[:, g, ko, n0:n0 + nw]),
                                first_extra=(lambda ops=ops, rows=rows, n0=n0, nw=nw:
                                             nc.tensor.matmul(
                                                 ops[:rows, 0:nw],
                                                 ones_row[0:1, 0:rows],
                                                 bo_row[0:1, n0:n0 + nw],
                                                 start=False, stop=True)))
                            nc.vector.tensor_tensor(x2t[:rows, i, n0:n0 + nw],
                                                    ops[:rows, 0:nw],
                                                    x_sb[:rows, i, n0:n0 + nw], OP.add)
                        nc.sync.dma_start(x2s[base + t0: base + t0 + rows, :],
                                          x2t[:rows, i, :])

                # software-pipelined emission: batch b+1's loads/LN/QKV are
                # emitted before batch b's attention so the scheduler always
                # has independent PE work during the ACT-bound softmax.
                emit_weights()
                prev = None
                for b in range(bpc):
                    st = emit_ab(b)
                    if prev is not None:
                        emit_cd(b - 1, prev)
                    prev = st
                emit_cd(bpc - 1, prev)

            # ================= phase 2: MLP (bf16, per chunk) =================
            with contextlib.ExitStack() as p2:
                pf1 = p2.enter_context(tc.tile_pool(name="pf1", bufs=3, space="PSUM"))
                pf2 = p2.enter_context(tc.tile_pool(name="pf2", bufs=3, space="PSUM"))
                ptp = p2.enter_context(tc.tile_pool(name="ptp2", bufs=2, space="PSUM"))
                stage = p2.enter_context(tc.tile_pool(name="stage2", bufs=2))
                w1pool = p2.enter_context(tc.tile_pool(name="w1pool", bufs=1))
                w2pool = p2.enter_context(tc.tile_pool(name="w2pool", bufs=1))
                x2cpool = p2.enter_context(tc.tile_pool(name="x2cpool", bufs=2))
                h2pool = p2.enter_context(tc.tile_pool(name="h2pool", bufs=2))
                hnpool = p2.enter_context(tc.tile_pool(name="hnpool2", bufs=2))
                mpool = p2.enter_context(tc.tile_pool(name="mpool", bufs=1))
                opool = p2.enter_context(tc.tile_pool(name="opool", bufs=2))

                cmax = max(cw for _, cw in chunks)
                ntile_max = (cmax + 127) // 128

                w1_sb = w1pool.tile([128, KK, MFF, 128], BF16, name="w1_sb")
                for kk in range(KK):
                    for half in range(2):
                        st = stage.tile([128, FF // 2], F32, tag="stage", name="w1_st")
                        nc.sync.dma_start(
                            st, w1[128 * kk:128 * (kk + 1),
                                   (FF // 2) * half:(FF // 2) * (half + 1)])
                        nc.vector.tensor_copy(
                            w1_sb[:, kk, 12 * half:12 * (half + 1), :]
                            .rearrange("p m e -> p (m e)"), st)
                # fc2 contraction split: m-tiles 0-11 run fp8-e4m3 DoubleRow,
                # 12-23 bf16; BOTH halves' weights are pre-scaled x64
                # (lossless for bf16) so they share one PSUM group, and the
                # 1/64 descale fuses into the residual-add epilogue.
                # Validated rel 0.0163 total vs 2e-2 budget.
                MF8 = 12 if USE_FP8 else 0
                G2 = MF8 // 2
                CP = (cmax + 15) // 16 * 16
                if USE_FP8:
                    w2_8 = w2pool.tile([128, G2, 2, D], FP8, name="w2_8")
                w2_sb = w2pool.tile([128, MFF - MF8, D], BF16, name="w2_sb")
                for m in range(MFF):
                    st = stage.tile([128, D], F32, tag="stage", name="w2_st")
                    nc.sync.dma_start(st, w2[128 * m:128 * (m + 1), :])
                    if m < MF8:
                        nc.vector.tensor_scalar_mul(w2_8[:, m // 2, m % 2, :],
                                                    st, 64.0)
                    else:
                        nc.vector.tensor_scalar_mul(w2_sb[:, m - MF8, :], st, 64.0)

                def emit_ln2(c0, cw):
                    ctiles = [(i0, min(128, cw - i0)) for i0 in range(0, cw, 128)]
                    x2c = x2cpool.tile([128, ntile_max, D], F32, name="x2c")
                    h2T = h2pool.tile([128, KK, cmax], BF16, name="h2T")
                    mvb = small.tile([128, ntile_max, 2], F32, tag="mvb", name="mvb2")
                    nc.vector.memset(mvb, 1.0)
                    for i, (i0, rows) in enumerate(ctiles):
                        nc.sync.dma_start(x2c[:rows, i, :],
                                          x2s[c0 + i0: c0 + i0 + rows, :])
                        _ln_stats_tile(nc, small, x2c[:rows, i, :], rows, mvb, i)
                    rstd_b = _rsqrt_batch(nc, small, mvb, len(ctiles))
                    for i, (i0, rows) in enumerate(ctiles):
                        hn = hnpool.tile([128, D], BF16, tag="hn", name="hn2")
                        _ln_apply(nc, x2c[:rows, i, :], rows, mvb, rstd_b, i, hn[:rows])
                        _transpose_to(nc, ptp, ident, hn, rows, h2T, i0,
                                      ln_pps["ln2g"], ln_pps["ln2b"], cw)
                    return x2c, h2T

                def emit_mlp(c0, cw, x2c, h2T):
                    ctiles = [(i0, min(128, cw - i0)) for i0 in range(0, cw, 128)]
                    cchunks = [(n0, min(512, cw - n0)) for n0 in range(0, cw, 512)]
                    if USE_FP8:
                        m_sb8 = mpool.tile([128, G2, 2, CP], FP8, tag="m8",
                                           name="m_sb8")
                    m_sb = mpool.tile([128, MFF - MF8, cmax], BF16, name="m_sb")
                    for m in range(MFF):
                        for n0, nw in cchunks:
                            fps = pf1.tile([128, 512], F32, tag="f1", name="fc1_ps")
                            for kk in range(KK):
                                nc.tensor.matmul(fps[:, 0:nw], w1_sb[:, kk, m, :],
                                                 h2T[:, kk, n0:n0 + nw],
                                                 start=(kk == 0), stop=(kk == KK - 1))
                            gdst = (m_sb8[:, m // 2, m % 2, n0:n0 + nw] if m < MF8
                                    else m_sb[:, m - MF8, n0:n0 + nw])
                            nc.scalar.activation(gdst, fps[:, 0:nw],
                                                 AF.Gelu_apprx_tanh,
                                                 bias=b1_pp[:, m:m + 1], scale=1.0)
                    for i, (i0, rows) in enumerate(ctiles):
                        ot = opool.tile([128, D], F32, tag="ot", name="ot")
                        for n0, nw in D_CHUNKS:
                            gps = pf2.tile([128, 512], F32, tag="f2", name="fc2_ps")
                            for g in range(G2):
                                nc.tensor.matmul(gps[:rows, 0:nw],
                                                 m_sb8[:, g, :, i0:i0 + rows],
                                                 w2_8[:, g, :, n0:n0 + nw],
                                                 start=(g == 0), stop=False,
                                                 perf_mode=DR)
                            for m in range(MFF - MF8):
                                nc.tensor.matmul(gps[:rows, 0:nw],
                                                 m_sb[:, m, i0:i0 + rows],
                                                 w2_sb[:, m, n0:n0 + nw],
                                                 start=(G2 == 0 and m == 0),
                                                 stop=False)
                            nc.tensor.matmul(gps[:rows, 0:nw],
                                             ones_row[0:1, 0:rows],
                                             b2_row[0:1, n0:n0 + nw],
                                             start=False, stop=True)
                            nc.vector.scalar_tensor_tensor(
                                ot[:rows, n0:n0 + nw], gps[:rows, 0:nw],
                                1.0 / 64.0, x2c[:rows, i, n0:n0 + nw],
                                OP.mult, OP.add)
                        nc.sync.dma_start(out[c0 + i0: c0 + i0 + rows, :], ot[:rows, :])

                prevc = None
                for c0, cw in chunks:
                    st2 = emit_ln2(c0, cw)
                    if prevc is not None:
                        emit_mlp(*prevc)
                    prevc = (c0, cw, *st2)
                emit_mlp(*prevc)
    return nc


_NC_CACHE = {}


def build_nc(bpc=B // NCORES):
    if bpc not in _NC_CACHE:
        from concourse import bacc
        nc = bacc.Bacc("TRN2", target_bir_lowering=False, debug=False)
        build_block(nc, bpc)
        nc.compile()
        _NC_CACHE[bpc] = nc
    return _NC_CACHE[bpc]


def run(inputs, **spmd_kwargs):
    from concourse.bass_utils import run_bass_kernel_spmd

    inputs = {k: np.ascontiguousarray(np.asarray(v, dtype=np.float32))
              for k, v in inputs.items()}
    x_full = inputs["x"]
    bpc = B // NCORES
    nc = build_nc(bpc)
    weights = {k: v for k, v in inputs.items() if k != "x"}
    in_maps = [dict(weights, x=np.ascontiguousarray(x_full[c * bpc:(c + 1) * bpc]))
               for c in range(NCORES)]
    res = run_bass_kernel_spmd(nc, in_maps, core_ids=list(range(NCORES)),
                               **spmd_kwargs)
    out = np.concatenate([r["out"] for r in res.results], axis=0)
    return out, res


def kernel(**inputs):
    return run(inputs)[0]


# revision 55
# speedup vs baseline: 1.0796x; 1.0550x over previous
"""Trainium2 Bass kernel: ViT-style dense transformer block (B=64,S=577,D=768,H=12).

Sharding: pure data-parallel over batch across 8 NeuronCores (8 batches/core,
no collectives).  Per core:

  Phase 1 (per batch): LN1 -> QKV -> attention -> out-proj + residual,
    spilling the residual stream x2 to DRAM scratch.
    - QKV and the out-projection run in fp8-e4m3 with DoubleRow perf mode
      (2 contraction rows / cycle).  Weights are pre-scaled (x16 for
      Wq/Wk/Wv with a 1/16 descale folded into the PSUM->SBUF epilogue;
      x4 for Wo with the 1/4 descale folded into the softmax denominator
      via a 4.0-valued ones column in V).
    - Scores are computed transposed (scoresT[j,i] = k_j . q_i) per head
      with K=64; the even/odd heads of a pair are issued back-to-back so
      they run concurrently in disjoint PE row-groups.
    - Softmax denominator comes from the extra 4.0 column appended to V;
      normalization = reciprocal + gpsimd partition-broadcast + multiply.
    - Biases bo (and bv) are folded into the matmul accumulation as K=1
      ones-row matmuls; bq/bk ride the PSUM->SBUF epilogues.

  Phase 2 (per 512-token chunk, last chunk 520): LN2 -> fc1 + tanh-GELU
    (scalar engine) -> fc2 + residual, all bf16 (fp8 fails the accuracy
    budget for the MLP), b2 folded into the fc2 accumulation.

PSUM plan (8 banks), phase 1: scores ring2 x 2 banks, PV accumulator
ring1 x 2 banks, and a shared ring2 of 1-bank tiles for QKV / out-proj
chunks + PE transposes — so next-batch QKV always has PSUM available
while the current batch's ACT-bound softmax runs.
"""

import math
import numpy as np

import concourse.bass as bass
import concourse.mybir as mybir
import concourse.tile as tile
from concourse.masks import make_identity

F32 = mybir.dt.float32
I32 = mybir.dt.int32
BF16 = mybir.dt.bfloat16
FP8 = mybir.dt.float8e4
AF = mybir.ActivationFunctionType
OP = mybir.AluOpType
DR = mybir.MatmulPerfMode.DoubleRow
RSQRT_MAGIC = 0x5F3759DF

B, S, D, H, DH = 64, 577, 768, 12, 64
SP = 592               # S padded so fp8 DoubleRow ko-steps are 16B-aligned
FF = 4 * D
EPS = 1e-6
NCORES = 8
KK = D // 128          # 6 k-tiles over D
G = KK // 2            # 3 DoubleRow k-groups
MFF = FF // 128        # 24 tiles over FF
NHP = H // 2           # 6 head pairs
SCALE = 1.0 / math.sqrt(DH)
WS_QKV = 16.0          # fp8 weight pre-scale for Wq/Wk/Wv
WS_O = 4.0             # fp8 weight pre-scale for Wo (descale via V ones col)

USE_FP8 = True

# token tiles within one sequence: 4 x 128 + 65
S_TILES = [(i * 128, min(128, S - i * 128)) for i in range((S + 127) // 128)]
# n-chunks over S and D for PSUM-bank-sized matmul outputs
S_CHUNKS = [(0, 512), (512, S - 512)]
D_CHUNKS = [(0, 512), (512, D - 512)]


def _bcast(ap):
    """[N] dram AP -> [128, N] partition-broadcast AP."""
    return bass.AP(tensor=ap.tensor, offset=ap.offset, ap=[[0, 128]] + list(ap.ap))


def _ln_stats_tile(nc, pool, x_sl, rows, mvb, i):
    """bn stats over the free dim (768) of x_sl[:rows] -> mvb[:, i, :]=(mu,var)."""
    stats = pool.tile([128, 3, 6], F32, tag="lnstats", name="lnstats")
    for sg in range(3):
        nc.vector.bn_stats(stats[:rows, sg, :], x_sl[:, 256 * sg:256 * (sg + 1)])
    nc.vector.bn_aggr(mvb[:rows, i, :], stats[:rows])


def _rsqrt_batch(nc, pool, mvb, n):
    """rstd[:, i] = 1/sqrt(var_i + EPS), magic-constant + 2 Newton iters on DVE."""
    veps = pool.tile([128, 8], F32, tag="lnveps", name="veps")
    nc.vector.tensor_scalar_add(veps[:, :n], mvb[:, 0:n, 1], EPS)
    hv = pool.tile([128, 8], F32, tag="lnhv", name="hv")
    nc.vector.tensor_scalar_mul(hv[:, :n], veps[:, :n], 0.5)
    y = pool.tile([128, 8], F32, tag="lnrstd", name="rstd_b")
    t = pool.tile([128, 8], F32, tag="lnnt", name="nt")
    nc.vector.tensor_scalar(t[:, :n].bitcast(I32), veps[:, :n].bitcast(I32),
                            1, None, op0=OP.arith_shift_right)
    nc.vector.tensor_scalar(y[:, :n].bitcast(I32), t[:, :n].bitcast(I32),
                            -1, RSQRT_MAGIC, op0=OP.mult, op1=OP.add)
    for _ in range(2):
        nc.vector.tensor_tensor(t[:, :n], y[:, :n], y[:, :n], OP.mult)
        nc.vector.tensor_tensor(t[:, :n], t[:, :n], hv[:, :n], OP.mult)
        nc.vector.tensor_scalar(t[:, :n], t[:, :n], -1.0, 1.5,
                                op0=OP.mult, op1=OP.add)
        nc.vector.tensor_tensor(y[:, :n], y[:, :n], t[:, :n], OP.mult)
    return y


def _ln_apply(nc, x_sl, rows, mvb, rstd_b, i, out_sl):
    """(x - mu_i) * rstd_i -> out_sl (one DVE op, per-partition scalars)."""
    nc.vector.tensor_scalar(out_sl, x_sl, mvb[:rows, i, 0:1],
                            rstd_b[:rows, i:i + 1],
                            op0=OP.subtract, op1=OP.mult)


def _transpose_to(nc, ptp, ident, src, rows, dst, col0, g_pp, b_pp, ncols):
    """src[:rows, 0:768] bf16 -> dst[:, kk, col0:col0+rows] feature-major,
    applying the LN gain/bias as per-partition scalars during the DVE copy."""
    for kk in range(KK):
        tp = ptp.tile([128, 128], BF16, tag="mm1", name="tp")
        nc.tensor.transpose(tp[:, :rows], src[:rows, 128 * kk:128 * (kk + 1)],
                            ident[:rows, :rows])
        nc.vector.tensor_scalar(dst[:, kk, col0:col0 + rows], tp[:, :rows],
                                g_pp[:, kk:kk + 1], b_pp[:, kk:kk + 1],
                                op0=OP.mult, op1=OP.add)


def build_block(nc: bass.Bass, bpc: int):
    tok = bpc * S
    # phase-2 chunks: 512-token chunks, tail merged into the last one
    nch = max(1, tok // 512)
    chunks = [(c0, 512) for c0 in range(0, (nch - 1) * 512, 512)]
    chunks.append(((nch - 1) * 512, tok - (nch - 1) * 512))

    x = nc.dram_tensor("x", [bpc, S, D], F32, kind="ExternalInput").ap().flatten_outer_dims()
    ln1_g = nc.dram_tensor("ln1_g", [D], F32, kind="ExternalInput").ap()
    ln1_b = nc.dram_tensor("ln1_b", [D], F32, kind="ExternalInput").ap()
    wq = nc.dram_tensor("Wq", [H, D, DH], F32, kind="ExternalInput").ap()
    bq = nc.dram_tensor("bq", [H, DH], F32, kind="ExternalInput").ap()
    wk = nc.dram_tensor("Wk", [H, D, DH], F32, kind="ExternalInput").ap()
    bk = nc.dram_tensor("bk", [H, DH], F32, kind="ExternalInput").ap()
    wv = nc.dram_tensor("Wv", [H, D, DH], F32, kind="ExternalInput").ap()
    bv = nc.dram_tensor("bv", [H, DH], F32, kind="ExternalInput").ap()
    wo = nc.dram_tensor("Wo", [D, D], F32, kind="ExternalInput").ap()
    bo = nc.dram_tensor("bo", [D], F32, kind="ExternalInput").ap()
    ln2_g = nc.dram_tensor("ln2_g", [D], F32, kind="ExternalInput").ap()
    ln2_b = nc.dram_tensor("ln2_b", [D], F32, kind="ExternalInput").ap()
    w1 = nc.dram_tensor("W1", [D, FF], F32, kind="ExternalInput").ap()
    b1 = nc.dram_tensor("b1", [FF], F32, kind="ExternalInput").ap()
    w2 = nc.dram_tensor("W2", [FF, D], F32, kind="ExternalInput").ap()
    b2 = nc.dram_tensor("b2", [D], F32, kind="ExternalInput").ap()
    out = nc.dram_tensor("out", [bpc, S, D], F32, kind="ExternalOutput").ap().flatten_outer_dims()
    x2s = nc.dram_tensor("x2_scratch", [tok, D], F32, kind="Internal").ap()

    w8 = FP8 if USE_FP8 else BF16
    sq, so = (WS_QKV, WS_O) if USE_FP8 else (1.0, 1.0)

    with tile.TileContext(nc) as tc:
        import contextlib
        with contextlib.ExitStack() as res:
            # ---------------- resident constants ----------------
            singles = res.enter_context(tc.tile_pool(name="singles", bufs=1))
            small = res.enter_context(tc.tile_pool(name="small", bufs=4))

            ident = singles.tile([128, 128], BF16, name="ident")
            make_identity(nc, ident)
            ones_row = singles.tile([1, 128], BF16, name="ones_row")
            nc.vector.memset(ones_row, 1.0)
            ones_s = singles.tile([1, S], BF16, name="ones_s")
            nc.vector.memset(ones_s, 1.0)

            # bias rows for K=1 fold-in matmuls
            def load_row_bf16(src_ap, name, scale=1.0):
                row = bass.AP(tensor=src_ap.tensor, offset=src_ap.offset,
                              ap=[[0, 1]] + list(src_ap.ap))
                st = singles.tile([1, D], F32, tag="rowstage", name="rowst")
                nc.sync.dma_start(st, row)
                t = singles.tile([1, D], BF16, name=name)
                nc.vector.tensor_scalar_mul(t, st, scale)
                return t

            bo_row = load_row_bf16(bo, "bo_row")
            b2_row = load_row_bf16(b2, "b2_row", 64.0)
            bv_row = load_row_bf16(bv.rearrange("h e -> (h e)"), "bv_row", sq)

            # per-partition biases / LN gain+bias in feature-major layout
            bq_pp = singles.tile([128, NHP], F32, name="bq_pp")
            nc.gpsimd.dma_start(bq_pp, bq.rearrange("(hp two) e -> (two e) hp", two=2))
            bk_pp = singles.tile([128, NHP], F32, name="bk_pp")
            nc.gpsimd.dma_start(bk_pp, bk.rearrange("(hp two) e -> (two e) hp", two=2))
            b1_pp = singles.tile([128, MFF], F32, name="b1_pp")
            nc.gpsimd.dma_start(b1_pp, b1.rearrange("(m p) -> p m", p=128))
            ln_pps = {}
            for nm, src in (("ln1g", ln1_g), ("ln1b", ln1_b),
                            ("ln2g", ln2_g), ("ln2b", ln2_b)):
                t = singles.tile([128, KK], F32, name=f"{nm}_pp")
                nc.gpsimd.dma_start(t, src.rearrange("(kk p) -> p kk", p=128))
                ln_pps[nm] = t

            # ================= phase 1: attention (per batch) =================
            with contextlib.ExitStack() as p1:
                stage = p1.enter_context(tc.tile_pool(name="stage1", bufs=2))
                wpool = p1.enter_context(tc.tile_pool(name="wpool1", bufs=1))

                # --- attention weights, fp8 DoubleRow layout [128, G, 2, ...] ---
                wq_sbs = [wpool.tile([128, G, 2, 128], w8, name=f"wq_sb{hp}")
                          for hp in range(NHP)]
                wk_sbs = [wpool.tile([128, G, 2, 128], w8, name=f"wk_sb{hp}")
                          for hp in range(NHP)]
                wv_sb = wpool.tile([128, G, 2, D], w8, name="wv_sb")
                wo_sb = wpool.tile([128, G, 2, D], w8, name="wo_sb")

                def emit_weights():
                    for h in range(H):
                        for dsts, wsrc in ((wq_sbs, wq), (wk_sbs, wk)):
                            hp, par = h // 2, h % 2
                            st = stage.tile([128, G, 2, DH], F32, tag="stage",
                                            name="wqk_st")
                            nc.sync.dma_start(
                                st, wsrc[h].rearrange("(g ko p) e -> p g ko e",
                                                      g=G, ko=2))
                            nc.vector.tensor_scalar_mul(
                                dsts[hp][:, :, :, DH * par:DH * par + DH], st, sq)
                    for h in range(H):
                        st = stage.tile([128, G, 2, DH], F32, tag="stage", name="wv_st")
                        nc.sync.dma_start(
                            st, wv[h].rearrange("(g ko p) e -> p g ko e", g=G, ko=2))
                        nc.vector.tensor_scalar_mul(
                            wv_sb[:, :, :, DH * h:DH * h + DH], st, sq)
                    for g in range(G):
                        for ko in range(2):
                            st = stage.tile([128, D], F32, tag="stage", name="wo_st")
                            kk = 2 * g + ko
                            nc.sync.dma_start(st, wo[128 * kk:128 * (kk + 1), :])
                            nc.vector.tensor_scalar_mul(wo_sb[:, g, ko, :], st, so)

                # PSUM pools: see module docstring
                pmm = p1.enter_context(tc.tile_pool(name="pmm", bufs=2, space="PSUM"))
                psc = p1.enter_context(tc.tile_pool(name="psc", bufs=2, space="PSUM"))
                pav = p1.enter_context(tc.tile_pool(name="pav", bufs=1, space="PSUM"))

                xpool = p1.enter_context(tc.tile_pool(name="xpool", bufs=2))
                x2pool = p1.enter_context(tc.tile_pool(name="x2pool", bufs=2))
                hnpool = p1.enter_context(tc.tile_pool(name="hnpool", bufs=3))
                h1pool = p1.enter_context(tc.tile_pool(name="h1pool", bufs=2))
                qkpool = p1.enter_context(tc.tile_pool(name="qkpool", bufs=2))
                vpool = p1.enter_context(tc.tile_pool(name="vpool", bufs=2))
                atpool = p1.enter_context(tc.tile_pool(name="atpool", bufs=2))
                epool = p1.enter_context(tc.tile_pool(name="epool", bufs=2))
                aupool = p1.enter_context(tc.tile_pool(name="aupool", bufs=5))

                def qkv_mm(ps, w_ap3, x_ap3, first_extra=None):
                    """Accumulate over G DoubleRow groups (or 2G bf16 k-tiles)."""
                    last = first_extra is None
                    if USE_FP8:
                        for g in range(G):
                            nc.tensor.matmul(ps, w_ap3(g), x_ap3(g),
                                             start=(g == 0),
                                             stop=(last and g == G - 1),
                                             perf_mode=DR)
                    else:
                        for g in range(G):
                            for ko in range(2):
                                nc.tensor.matmul(ps, w_ap3(g, ko), x_ap3(g, ko),
                                                 start=(g == 0 and ko == 0),
                                                 stop=(last and g == G - 1 and ko == 1))
                    if first_extra is not None:
                        first_extra()

                def emit_ab(b):
                    base = b * S
                    # ---- A: load x, LN1, transpose to feature-major fp8 ----
                    x_sb = xpool.tile([128, len(S_TILES), D], F32, name="x_sb")
                    h1T = h1pool.tile([128, KK, SP], w8, name="h1T")
                    mvb = small.tile([128, len(S_TILES), 2], F32, tag="mvb", name="mvb")
                    nc.vector.memset(mvb, 1.0)
                    for i, (t0, rows) in enumerate(S_TILES):
                        nc.sync.dma_start(x_sb[:rows, i, :], x[base + t0: base + t0 + rows, :])
                        _ln_stats_tile(nc, small, x_sb[:rows, i, :], rows, mvb, i)
                    rstd_b = _rsqrt_batch(nc, small, mvb, len(S_TILES))
                    for i, (t0, rows) in enumerate(S_TILES):
                        hn = hnpool.tile([128, D], BF16, tag="hn", name="hn")
                        _ln_apply(nc, x_sb[:rows, i, :], rows, mvb, rstd_b, i, hn[:rows])
                        _transpose_to(nc, pmm, ident, hn, rows, h1T, t0,
                                      ln_pps["ln1g"], ln_pps["ln1b"], S)

                    # ---- B: QKV (fp8 DoubleRow) ----
                    # per-hp q/k tiles: pair-0 attention starts as soon as its
                    # own epilogues land (helps the batch-0 pipeline fill)
                    q_sb = [qkpool.tile([128, S], BF16, name=f"q_sb{hp}")
                            for hp in range(NHP)]
                    k_sb = [qkpool.tile([128, S], BF16, name=f"k_sb{hp}")
                            for hp in range(NHP)]
                    for hp in range(NHP):
                        for which, (dst, wsb, bpp) in enumerate(
                                ((q_sb[hp], wq_sbs, bq_pp), (k_sb[hp], wk_sbs, bk_pp))):
                            for n0, nw in S_CHUNKS:
                                ps = pmm.tile([128, 512], F32, tag="mm1", name="qk_ps")
                                qkv_mm(
                                    ps[:, 0:nw],
                                    (lambda g, ko=None, n0=n0, nw=nw:
                                     wsb[hp][:, g, :, :] if ko is None
                                     else wsb[hp][:, g, ko, :]),
                                    (lambda g, ko=None, n0=n0, nw=nw:
                                     h1T[:, 2 * g:2 * g + 2, n0:n0 + nw] if ko is None
                                     else h1T[:, 2 * g + ko, n0:n0 + nw]))
                                if which == 0:
                                    nc.scalar.activation(
                                        dst[:, n0:n0 + nw], ps[:, 0:nw],
                                        AF.Identity, bias=bpp[:, hp:hp + 1],
                                        scale=1.0 / sq)
                                else:
                                    nc.vector.tensor_scalar(
                                        dst[:, n0:n0 + nw], ps[:, 0:nw],
                                        1.0 / sq, bpp[:, hp:hp + 1],
                                        op0=OP.mult, op1=OP.add)
                    # v kept at the x{sq} weight scale; the ones column carries
                    # sq*WS_O so the softmax normalization descales v AND Wo.
                    v_aug = vpool.tile([128, len(S_TILES), H, DH + 1], BF16, name="v_aug")
                    for i, (t0, rows) in enumerate(S_TILES):
                        for n0, nw in D_CHUNKS:
                            ps = pmm.tile([128, 512], F32, tag="mm1", name="v_ps")
                            qkv_mm(
                                ps[:rows, 0:nw],
                                (lambda g, ko=None, n0=n0, nw=nw:
                                 h1T[:, 2 * g:2 * g + 2, t0:t0 + rows] if ko is None
                                 else h1T[:, 2 * g + ko, t0:t0 + rows]),
                                (lambda g, ko=None, n0=n0, nw=nw:
                                 wv_sb[:, g, :, n0:n0 + nw] if ko is None
                                 else wv_sb[:, g, ko, n0:n0 + nw]),
                                first_extra=(lambda ps=ps, rows=rows, n0=n0, nw=nw:
                                             nc.tensor.matmul(
                                                 ps[:rows, 0:nw],
                                                 ones_row[0:1, 0:rows],
                                                 bv_row[0:1, n0:n0 + nw],
                                                 start=False, stop=True)))
                            nc.vector.tensor_copy(
                                v_aug[:rows, i, n0 // DH:(n0 + nw) // DH, 0:DH],
                                ps[:rows, 0:nw].rearrange("p (h e) -> p h e", e=DH))
                        nc.vector.memset(v_aug[:rows, i, :, DH:DH + 1], sq * so)

                    return dict(x_sb=x_sb, h1T=h1T, q_sb=q_sb, k_sb=k_sb,
                                v_aug=v_aug)

                def emit_cd(b, st):
                    base = b * S
                    x_sb, q_sb, k_sb, v_aug = (st["x_sb"], st["q_sb"],
                                               st["k_sb"], st["v_aug"])
                    # ---- C: attention per head pair ----
                    attnT = atpool.tile([128, KK, SP], w8, name="attnT")
                    for hp in range(NHP):
                        expT = epool.tile([128, len(S_TILES), 2, S], BF16,
                                          tag="expT", name="expT")
                        for j, (t0, rj) in enumerate(S_TILES):
                            sps_t = {}
                            # even/odd scores adjacent -> concurrent row-groups
                            for par in range(2):
                                off = DH * par
                                sps = psc.tile([128, S], F32, tag="sps", name="sc_ps")
                                sps_t[par] = sps
                                for n0, nw in S_CHUNKS:
                                    nc.tensor.matmul(sps[:rj, n0:n0 + nw],
                                                     k_sb[hp][off:off + DH, t0:t0 + rj],
                                                     q_sb[hp][off:off + DH, n0:n0 + nw],
                                                     start=True, stop=True)
                            for par in range(2):
                                nc.scalar.activation(expT[:rj, j, par, :],
                                                     sps_t[par][:rj, :],
                                                     AF.Exp, bias=0.0, scale=SCALE)
                        for par in range(2):
                            h = 2 * hp + par
                            off = DH * par
                            aps = pav.tile([DH + 1, S], F32, tag="aps", name="attn_ps")
                            for n0, nw in S_CHUNKS:
                                for j, (t0, rj) in enumerate(S_TILES):
                                    nc.tensor.matmul(aps[:, n0:n0 + nw],
                                                     v_aug[:rj, j, h, :],
                                                     expT[:rj, j, par, n0:n0 + nw],
                                                     start=(j == 0),
                                                     stop=(j == len(S_TILES) - 1))
                            # evacuate PSUM immediately (ACT) so the next head's
                            # PV can start; normalize from SBUF off the chain
                            att_un = aupool.tile([DH + 1, S], BF16, tag="attun",
                                                 name="att_un")
                            nc.scalar.activation(att_un, aps, AF.Identity)
                            rec = small.tile([1, S], BF16, tag="rec", name="rec")
                            with nc.allow_low_precision(reason="softmax denom bf16"):
                                nc.vector.reciprocal(rec, att_un[DH:DH + 1, :])
                            rec_bc = small.tile([DH, S], BF16, tag="recbc", name="rec_bc")
                            nc.gpsimd.partition_broadcast(rec_bc, rec, channels=DH)
                            nc.vector.tensor_tensor(attnT[off:off + DH, hp, 0:S],
                                                    att_un[0:DH, :], rec_bc, OP.mult)

                    # ---- D: out-proj (fp8 DoubleRow) + residual -> DRAM ----
                    x2t = x2pool.tile([128, len(S_TILES), D], F32, name="x2t")
                    for i, (t0, rows) in enumerate(S_TILES):
                        for n0, nw in D_CHUNKS:
                            ops = pmm.tile([128, 512], F32, tag="mm1", name="op_ps")
                            qkv_mm(
                                ops[:rows, 0:nw],
                                (lambda g, ko=None, t0=t0, rows=rows:
                                 attnT[:, 2 * g:2 * g + 2, t0:t0 + rows] if ko is None
                                 else attnT[:, 2 * g + ko, t0:t0 + rows]),
                                (lambda g, ko=None, n0=n0, nw=nw:
                                 wo_sb[:, g, :, n0:n0 + nw] if ko is None
                                 else wo_sb[:, g, ko, n0:n0 + nw]),
                                first_extra=(lambda ops=ops, rows=rows, n0=n0, nw=nw:
                                             nc.tensor.matmul(
                                                 ops[:rows, 0:nw],
                                                 ones_row[0:1, 0:rows],
                                                 bo_row[0:1, n0:n0 + nw],
                                                 start=False, stop=True)))
                            nc.vector.tensor_tensor(x2t[:rows, i, n0:n0 + nw],
                                                    ops[:rows, 0:nw],
                                                    x_sb[:rows, i, n0:n0 + nw], OP.add)
                        nc.sync.dma_start(x2s[base + t0: base + t0 + rows, :],
                                          x2t[:rows, i, :])

                # software-pipelined emission: batch b+1's loads/LN/QKV are
                # emitted before batch b's attention so the scheduler always
                # has independent PE work during the ACT-bound softmax.
                emit_weights()
                prev = None
                for b in range(bpc):
                    st = emit_ab(b)
                    if prev is not None:
                        emit_cd(b - 1, prev)
                    prev = st
                emit_cd(bpc - 1, prev)

            # ================= phase 2: MLP (bf16, per chunk) =================
            with contextlib.ExitStack() as p2:
                pf1 = p2.enter_context(tc.tile_pool(name="pf1", bufs=3, space="PSUM"))
                pf2 = p2.enter_context(tc.tile_pool(name="pf2", bufs=3, space="PSUM"))
                ptp = p2.enter_context(tc.tile_pool(name="ptp2", bufs=2, space="PSUM"))
                stage = p2.enter_context(tc.tile_pool(name="stage2", bufs=2))
                w1pool = p2.enter_context(tc.tile_pool(name="w1pool", bufs=1))
                w2pool = p2.enter_context(tc.tile_pool(name="w2pool", bufs=1))
                x2cpool = p2.enter_context(tc.tile_pool(name="x2cpool", bufs=2))
                h2pool = p2.enter_context(tc.tile_pool(name="h2pool", bufs=2))
                hnpool = p2.enter_context(tc.tile_pool(name="hnpool2", bufs=2))
                mpool = p2.enter_context(tc.tile_pool(name="mpool", bufs=1))
                opool = p2.enter_context(tc.tile_pool(name="opool", bufs=2))

                cmax = max(cw for _, cw in chunks)
                ntile_max = (cmax + 127) // 128

                w1_sb = w1pool.tile([128, KK, MFF, 128], BF16, name="w1_sb")
                for kk in range(KK):
                    for half in range(2):
                        st = stage.tile([128, FF // 2], F32, tag="stage", name="w1_st")
                        nc.sync.dma_start(
                            st, w1[128 * kk:128 * (kk + 1),
                                   (FF // 2) * half:(FF // 2) * (half + 1)])
                        nc.vector.tensor_copy(
                            w1_sb[:, kk, 12 * half:12 * (half + 1), :]
                            .rearrange("p m e -> p (m e)"), st)
                # fc2 contraction split: m-tiles 0-11 run fp8-e4m3 DoubleRow,
                # 12-23 bf16; BOTH halves' weights are pre-scaled x64
                # (lossless for bf16) so they share one PSUM group, and the
                # 1/64 descale fuses into the residual-add epilogue.
                # Validated rel 0.0163 total vs 2e-2 budget.
                MF8 = 12 if USE_FP8 else 0
                G2 = MF8 // 2
                CP = (cmax + 15) // 16 * 16
                if USE_FP8:
                    w2_8 = w2pool.tile([128, G2, 2, D], FP8, name="w2_8")
                w2_sb = w2pool.tile([128, MFF - MF8, D], BF16, name="w2_sb")
                for m in range(MFF):
                    st = stage.tile([128, D], F32, tag="stage", name="w2_st")
                    nc.sync.dma_start(st, w2[128 * m:128 * (m + 1), :])
                    if m < MF8:
                        nc.vector.tensor_scalar_mul(w2_8[:, m // 2, m % 2, :],
                                                    st, 64.0)
                    else:
                        nc.vector.tensor_scalar_mul(w2_sb[:, m - MF8, :], st, 64.0)

                def emit_ln2(c0, cw):
                    ctiles = [(i0, min(128, cw - i0)) for i0 in range(0, cw, 128)]
                    x2c = x2cpool.tile([128, ntile_max, D], F32, name="x2c")
                    h2T = h2pool.tile([128, KK, cmax], BF16, name="h2T")
                    mvb = small.tile([128, ntile_max, 2], F32, tag="mvb", name="mvb2")
                    nc.vector.memset(mvb, 1.0)
                    for i, (i0, rows) in enumerate(ctiles):
                        nc.sync.dma_start(x2c[:rows, i, :],
                                          x2s[c0 + i0: c0 + i0 + rows, :])
                        _ln_stats_tile(nc, small, x2c[:rows, i, :], rows, mvb, i)
                    rstd_b = _rsqrt_batch(nc, small, mvb, len(ctiles))
                    for i, (i0, rows) in enumerate(ctiles):
                        hn = hnpool.tile([128, D], BF16, tag="hn", name="hn2")
                        _ln_apply(nc, x2c[:rows, i, :], rows, mvb, rstd_b, i, hn[:rows])
                        _transpose_to(nc, ptp, ident, hn, rows, h2T, i0,
                                      ln_pps["ln2g"], ln_pps["ln2b"], cw)
                    return x2c, h2T

                def emit_mlp(c0, cw, x2c, h2T):
                    ctiles = [(i0, min(128, cw - i0)) for i0 in range(0, cw, 128)]
                    cchunks = [(n0, min(512, cw - n0)) for n0 in range(0, cw, 512)]
                    if USE_FP8:
                        m_sb8 = mpool.tile([128, G2, 2, CP], FP8, tag="m8",
                                           name="m_sb8")
                    m_sb = mpool.tile([128, MFF - MF8, cmax], BF16, name="m_sb")
                    for m in range(MFF):
                        for n0, nw in cchunks:
                            fps = pf1.tile([128, 512], F32, tag="f1", name="fc1_ps")
                            for kk in range(KK):
                                nc.tensor.matmul(fps[:, 0:nw], w1_sb[:, kk, m, :],
                                                 h2T[:, kk, n0:n0 + nw],
                                                 start=(kk == 0), stop=(kk == KK - 1))
                            gdst = (m_sb8[:, m // 2, m % 2, n0:n0 + nw] if m < MF8
                                    else m_sb[:, m - MF8, n0:n0 + nw])
                            nc.scalar.activation(gdst, fps[:, 0:nw],
                                                 AF.Gelu_apprx_tanh,
                                                 bias=b1_pp[:, m:m + 1], scale=1.0)
                    for i, (i0, rows) in enumerate(ctiles):
                        ot = opool.tile([128, D], F32, tag="ot", name="ot")
                        for n0, nw in D_CHUNKS:
                            gps = pf2.tile([128, 512], F32, tag="f2", name="fc2_ps")
                            for g in range(G2):
                                nc.tensor.matmul(gps[:rows, 0:nw],
                                                 m_sb8[:, g, :, i0:i0 + rows],
                                                 w2_8[:, g, :, n0:n0 + nw],
                                                 start=(g == 0), stop=False,
                                                 perf_mode=DR)
                            for m in range(MFF - MF8):
                                nc.tensor.matmul(gps[:rows, 0:nw],
                                                 m_sb[:, m, i0:i0 + rows],
                                                 w2_sb[:, m, n0:n0 + nw],
                                                 start=(G2 == 0 and m == 0),
                                                 stop=False)
                            nc.tensor.matmul(gps[:rows, 0:nw],
                                             ones_row[0:1, 0:rows],
                                             b2_row[0:1, n0:n0 + nw],
                                             start=False, stop=True)
                            nc.vector.scalar_tensor_tensor(
                                ot[:rows, n0:n0 + nw], gps[:rows, 0:nw],
                                1.0 / 64.0, x2c[:rows, i, n0:n0 + nw],
                                OP.mult, OP.add)
                        nc.sync.dma_start(out[c0 + i0: c0 + i0 + rows, :], ot[:rows, :])

                prevc = None
                for c0, cw in chunks:
                    st2 = emit_ln2(c0, cw)
                    if prevc is not None:
                        emit_mlp(*prevc)
                    prevc = (c0, cw, *st2)
                emit_mlp(*prevc)
    return nc


_NC_CACHE = {}


def build_nc(bpc=B // NCORES):
    if bpc not in _NC_CACHE:
        from concourse import bacc
        nc = bacc.Bacc("TRN2", target_bir_lowering=False, debug=False)
        build_block(nc, bpc)
        nc.compile()
        _NC_CACHE[bpc] = nc
    return _NC_CACHE[bpc]


def run(inputs, **spmd_kwargs):
    from concourse.bass_utils import run_bass_kernel_spmd

    inputs = {k: np.ascontiguousarray(np.asarray(v, dtype=np.float32))
              for k, v in inputs.items()}
    x_full = inputs["x"]
    bpc = B // NCORES
    nc = build_nc(bpc)
    weights = {k: v for k, v in inputs.items() if k != "x"}
    in_maps = [dict(weights, x=np.ascontiguousarray(x_full[c * bpc:(c + 1) * bpc]))
               for c in range(NCORES)]
    res = run_bass_kernel_spmd(nc, in_maps, core_ids=list(range(NCORES)),
                               **spmd_kwargs)
    out = np.concatenate([r["out"] for r in res.results], axis=0)
    return out, res


def kernel(**inputs):
    return run(inputs)[0]
